# revision 1
# baseline (speedup 1.0000x reference)
# Trainium2 Bass kernel for nn_Decoder (LSTM decoder + GCN message passing).
#
# Strategy (8 NeuronCores, SPMD):
#   * Data-parallel over nodes N=10000 -> 1250 nodes/core for fc2 + LSTM +
#     projection. State kept feature-major ([H, nodes]) so every matmul is
#     PE-friendly with K=H=128 and no transposes.
#   * Algebraic rewrite: the GCN aggregation and fc3 are both linear, so
#     aggregate AFTER projecting features to NF=16:
#        x_hat[n,t] = dinv[n] * sum_{e: dst=n} (dinv[src] * mask[src] * hs[t,src] @ (W_gcn@W_fc3))
#                     + (b_gcn@W_fc3 + b_fc3)
#     This shrinks the scatter/gather payload 8x (H=128 -> NF=16 per t).
#   * Y table ([N, T*NF], fp16, dinv*mask pre-scaled) is AllGather'ed across
#     the 8 cores. The scatter-add over edges is reformulated as a
#     block-sparse matmul: the host densifies the normalized adjacency into
#     128x128 blocks A[dst_tile, src_blk] (entry = edge multiplicity), and
#     each core computes agg[dst_tile] = sum_sb A[dst_tile,sb].T @ Y[sb]
#     streaming Y sequentially (direct DMA only - indirect DMA's per-row
#     descriptor generation on GpSimd would cost ~200us).
import os
import numpy as np

import concourse.bass as bass
import concourse.bacc as bacc
import concourse.tile as tile
from concourse import mybir
from concourse import bass_utils

P = 128
N, T, NF, H, L, E = 10000, 12, 16, 128, 64, 160000
NCORES = 8
NCN = N // NCORES            # 1250 nodes per core
NTILES = (NCN + P - 1) // P  # 10 dst tiles per core
NSB = (N + P - 1) // P       # 79 source blocks (last has 16 rows)
CH = [(0, 512), (512, 512), (1024, NCN - 1024)]  # LSTM node chunks (<=512)
TNF = T * NF                 # 192

F32 = mybir.dt.float32
F16 = mybir.dt.float16
U8 = mybir.dt.uint8

# gate q: 0=i, 1=f, 2=g, 3=o ; activation: sigmoid for i,f,o ; tanh for g
GATE_FUNCS = ["Sigmoid", "Sigmoid", "Tanh", "Sigmoid"]

_BUILD_CACHE = {}
LAST_RESULTS = None  # BassKernelResults of the most recent run (for test harness)


def _build():
    nc = bacc.Bacc("TRN2", target_bir_lowering=False, debug=False,
                   num_devices=NCORES)

    # ---------------- I/O declarations ----------------
    zT = nc.dram_tensor("zT", [L, NCN], F32, kind="ExternalInput")
    xm = nc.dram_tensor("xm", [NCN, TNF], U8, kind="ExternalInput")
    wfc2 = nc.dram_tensor("wfc2", [L, H], F32, kind="ExternalInput")
    b2 = nc.dram_tensor("b2", [P, 1], F32, kind="ExternalInput")
    wih = nc.dram_tensor("wih", [H, 4 * H], F16, kind="ExternalInput")
    whh = nc.dram_tensor("whh", [H, 4 * H], F16, kind="ExternalInput")
    bg = nc.dram_tensor("bg", [P, 4], F32, kind="ExternalInput")
    wcomb = nc.dram_tensor("wcomb", [H, NF], F16, kind="ExternalInput")
    bout = nc.dram_tensor("bout", [P, TNF], F32, kind="ExternalInput")
    dinvt = nc.dram_tensor("dinvt", [P, NTILES], F32, kind="ExternalInput")
    # A-blocks, wave-major: row (w*NSB + sb)*128 + p ; col = k_local*128 + drel
    ablk = nc.dram_tensor("ablk", [2 * NSB * P, 5 * P], F16,
                          kind="ExternalInput")
    xhat = nc.dram_tensor("xhat", [NCN, TNF], F32, kind="ExternalOutput")

    with tile.TileContext(nc) as tc:
        with tc.tile_pool(name="cpool", bufs=1) as cp, \
             tc.tile_pool(name="spool", bufs=1) as sp, \
             tc.tile_pool(name="dram", bufs=1, space="DRAM") as dp:

            # ---- constant loads ----
            zt_sb = cp.tile([L, NCN], F32)
            nc.sync.dma_start(zt_sb[:], zT[:])
            wfc2_sb = cp.tile([L, H], F32)
            nc.sync.dma_start(wfc2_sb[:], wfc2[:])
            b2_sb = cp.tile([P, 1], F32)
            nc.sync.dma_start(b2_sb[:], b2[:])
            wih_sb = cp.tile([H, 4 * H], F16)
            nc.sync.dma_start(wih_sb[:], wih[:])
            whh_sb = cp.tile([H, 4 * H], F16)
            nc.sync.dma_start(whh_sb[:], whh[:])
            bg_sb = cp.tile([P, 4], F32)
            nc.sync.dma_start(bg_sb[:], bg[:])
            wcomb_sb = cp.tile([H, NF], F16)
            nc.sync.dma_start(wcomb_sb[:], wcomb[:])
            bout_sb = cp.tile([P, TNF], F32)
            nc.sync.dma_start(bout_sb[:], bout[:])
            dinv_sb = cp.tile([P, NTILES], F32)
            nc.sync.dma_start(dinv_sb[:], dinvt[:])

            SL = [(0, 96), (96, 48), (144, 48)]  # (col0, width) per AG slice
            yshard_s = [dp.tile([NCN, w], F16, name=f"yshard{i}")
                        for i, (c0, w) in enumerate(SL)]
            yfull_s = [dp.tile([N, w], F16, addr_space="Shared",
                               name=f"yfull{i}")
                       for i, (c0, w) in enumerate(SL)]

            # ---- node mask * dinv (per node-block) ----
            mdv_sb = sp.tile([P, NTILES], F32)
            with tc.tile_pool(name="wp0", bufs=3) as wp0:
                for k in range(NTILES):
                    rows = min(P, NCN - k * P)
                    xmu = wp0.tile([P, TNF], U8, tag="xmu", bufs=3)
                    nc.sync.dma_start(xmu[:rows], xm[k * P:k * P + rows, :])
                    xmf = wp0.tile([P, TNF], F32, tag="xmf", bufs=3)
                    nc.vector.tensor_copy(out=xmf[:rows], in_=xmu[:rows])
                    mx = wp0.tile([P, 1], F32, tag="mx", bufs=3)
                    nc.vector.reduce_max(out=mx[:rows], in_=xmf[:rows],
                                         axis=mybir.AxisListType.X)
                    nc.vector.tensor_mul(out=mdv_sb[:rows, k:k + 1],
                                         in0=mx[:rows],
                                         in1=dinv_sb[:rows, k:k + 1])

            # ---- hd = z @ W_fc2 + b_fc2 (feature-major: hdT [H, nodes]) ----
            hdT = sp.tile([H, NCN], F16)
            with tc.tile_pool(name="psI", bufs=2, space="PSUM") as psI:
                for off, sz in CH:
                    ph = psI.tile([P, 512], F32, tag="ph", bufs=2)
                    nc.tensor.matmul(out=ph[:, :sz], lhsT=wfc2_sb[:],
                                     rhs=zt_sb[:, off:off + sz],
                                     start=True, stop=True)
                    nc.scalar.activation(
                        out=hdT[:, off:off + sz], in_=ph[:, :sz],
                        func=mybir.ActivationFunctionType.Identity,
                        bias=b2_sb[:, :1])

            # ---- LSTM (T steps, feature-major state) ----
            # Full-width (1250) PSUM per gate; weight loads ordered so each
            # of the 8 weight tiles is loaded once per step.
            cstate = sp.tile([P, NCN], F32)
            nc.vector.memset(cstate[:], 0.0)

            hs = []  # hs[t] tiles [H, NCN]
            hs_pool = tc.tile_pool(name="hspool", bufs=1)
            hsp = hs_pool.__enter__()
            ysb_t = [sp.tile([P, TNF], F16, name=f"ysb_{k}", tag=f"ysb_{k}")
                     for k in range(NTILES)]
            NFULL = NSB - 1  # 78 full source blocks, then a 16-row tail
            ytab = sp.tile([P, NSB * TNF], F16, name="ytab")

            def ship_slice(i):
                c0, w = SL[i]
                for k in range(NTILES):
                    rows = min(P, NCN - k * P)
                    nc.sync.dma_start(yshard_s[i][k * P:k * P + rows, :],
                                      ysb_t[k][:rows, c0:c0 + w])
                nc.gpsimd.collective_compute(
                    "AllGather", mybir.AluOpType.bypass,
                    replica_groups=[list(range(NCORES))],
                    ins=[yshard_s[i].opt()], outs=[yfull_s[i].opt()],
                )
                nc.sync.dma_start(
                    ytab[:, :NFULL * TNF].rearrange(
                        "p (sb f) -> p sb f", f=TNF)[:, :, c0:c0 + w],
                    yfull_s[i][:NFULL * P, :].rearrange(
                        "(sb p) f -> p sb f", p=P))
                nc.sync.dma_start(
                    ytab[:N - NFULL * P,
                         NFULL * TNF + c0:NFULL * TNF + c0 + w],
                    yfull_s[i][NFULL * P:, :])
            with tc.tile_pool(name="psG", bufs=2, space="PSUM") as psG, \
                 tc.tile_pool(name="psY", bufs=2, space="PSUM") as psY, \
                 tc.tile_pool(name="wpL", bufs=2) as wpL:
                def emit_proj(t):
                    for k in range(NTILES):
                        rows = min(P, NCN - k * P)
                        py = psY.tile([P, NF], F32, tag="py", bufs=2)
                        nc.tensor.matmul(out=py[:rows, :],
                                         lhsT=hs[t][:, k * P:k * P + rows],
                                         rhs=wcomb_sb[:],
                                         start=True, stop=True)
                        nc.vector.tensor_scalar(
                            out=ysb_t[k][:rows, t * NF:(t + 1) * NF],
                            in0=py[:rows, :],
                            scalar1=mdv_sb[:rows, k:k + 1],
                            scalar2=None, op0=mybir.AluOpType.mult)
                    if t == 5:
                        ship_slice(0)
                    elif t == 8:
                        ship_slice(1)

                for t in range(T):
                    prev = hdT if t == 0 else hs[t - 1]
                    sg = [None] * 4
                    pqs = [None] * 4

                    def emit_ih(q):
                        wsl = slice(q * H, (q + 1) * H)
                        pqs[q] = psG.tile([P, NCN], F32, name="pq", tag="pq", bufs=2)
                        for off, sz in CH:
                            nc.tensor.matmul(out=pqs[q][:, off:off + sz],
                                             lhsT=wih_sb[:, wsl],
                                             rhs=hdT[:, off:off + sz],
                                             start=True, stop=False)

                    def emit_hh_act(q):
                        wsl = slice(q * H, (q + 1) * H)
                        for off, sz in CH:
                            nc.tensor.matmul(out=pqs[q][:, off:off + sz],
                                             lhsT=whh_sb[:, wsl],
                                             rhs=prev[:, off:off + sz],
                                             start=False, stop=True)
                        sg[q] = wpL.tile([P, NCN], F32, name=f"sg{q}", tag=f"sg{q}", bufs=2)
                        nc.scalar.activation(
                            out=sg[q][:], in_=pqs[q][:],
                            func=getattr(mybir.ActivationFunctionType,
                                         GATE_FUNCS[q]),
                            bias=bg_sb[:, q:q + 1])

                    emit_ih(0)
                    emit_ih(1)
                    if t > 0:
                        emit_proj(t - 1)   # fills PE while h_{t-1} finishes
                    emit_hh_act(0)
                    emit_hh_act(1)
                    for q in (2, 3):
                        emit_ih(q)
                        emit_hh_act(q)

                    nc.vector.tensor_mul(out=cstate[:], in0=cstate[:],
                                         in1=sg[1][:])
                    tmp = wpL.tile([P, NCN], F32, tag="tmp", bufs=2)
                    nc.vector.tensor_mul(out=tmp[:], in0=sg[0][:], in1=sg[2][:])
                    nc.vector.tensor_add(out=cstate[:], in0=cstate[:],
                                         in1=tmp[:])
                    thc = wpL.tile([P, NCN], F32, tag="thc", bufs=2)
                    nc.scalar.activation(
                        out=thc[:], in_=cstate[:],
                        func=mybir.ActivationFunctionType.Tanh)
                    h_t = hsp.tile([P, NCN], F16, name=f"h_{t}", tag=f"h_{t}")
                    nc.vector.tensor_mul(out=h_t[:], in0=sg[3][:], in1=thc[:])
                    hs.append(h_t)
                emit_proj(T - 1)
                ship_slice(2)
            hs_pool.__exit__(None, None, None)  # release hs SBUF before GCN

            # ---- GCN aggregation: agg[k] = sum_sb A[k,sb].T @ Y[sb] ----
            # Whole Y table SBUF-resident: ytab[p, sb*192+f] = Y[sb*128+p, f].
            # A-blocks stream in 8-sb chunks. 2 waves of 5 dst tiles.
            with tc.tile_pool(name="psC", bufs=1, space="PSUM") as psC, \
                 tc.tile_pool(name="wpC", bufs=2) as wpC:
                CHUNK = 8
                sb_chunks = [(s, min(s + CHUNK, NSB))
                             for s in range(0, NSB, CHUNK)]
                for w, wave in enumerate((range(0, 5), range(5, NTILES))):
                    wave = list(wave)
                    pa = {k: psC.tile([P, TNF], F32, name=f"pa_{k}",
                                      tag=f"pa{i}", bufs=1)
                          for i, k in enumerate(wave)}
                    for (s0, s1) in sb_chunks:
                        nsb_c = s1 - s0
                        abc = wpC.tile([P, CHUNK * 5 * P], F16, tag="abc",
                                       bufs=3)
                        r0 = (w * NSB + s0) * P
                        r1 = (w * NSB + s1) * P
                        nc.gpsimd.dma_start(
                            abc[:, :nsb_c * 5 * P].rearrange(
                                "p (sb d) -> p sb d", d=5 * P),
                            ablk[r0:r1, :].rearrange("(sb p) d -> p sb d",
                                                     p=P))
                        for sb in range(s0, s1):
                            srows = min(P, N - sb * P)
                            aoff = (sb - s0) * 5 * P
                            for i, k in enumerate(wave):
                                nc.tensor.matmul(
                                    out=pa[k][:],
                                    lhsT=abc[:srows,
                                             aoff + i * P:aoff + (i + 1) * P],
                                    rhs=ytab[:srows,
                                             sb * TNF:(sb + 1) * TNF],
                                    start=(sb == 0),
                                    stop=(sb == NSB - 1))
                    for i, k in enumerate(wave):
                        rows = min(P, NCN - k * P)
                        osb = wpC.tile([P, TNF], F32, tag="osb", bufs=2)
                        nc.vector.tensor_scalar(out=osb[:rows],
                                                in0=pa[k][:rows],
                                                scalar1=dinv_sb[:rows, k:k + 1],
                                                scalar2=None,
                                                op0=mybir.AluOpType.mult)
                        nc.vector.tensor_add(out=osb[:rows], in0=osb[:rows],
                                             in1=bout_sb[:rows])
                        nc.sync.dma_start(xhat[k * P:k * P + rows, :],
                                          osb[:rows])

    nc.compile()
    return nc


def _preprocess(z, edge_index, x_mask, W_fc2, b_fc2, W_ih, W_hh, b_ih, b_hh,
                W_gcn, b_gcn, W_fc3, b_fc3):
    z = np.asarray(z, np.float32)
    edge_index = np.asarray(edge_index).astype(np.int64)
    x_mask = np.asarray(x_mask)
    src = edge_index[0]
    dst = edge_index[1]
    deg = (np.bincount(dst, minlength=N) + 1.0)
    dinv = (1.0 / np.sqrt(deg)).astype(np.float32)

    src_all = np.concatenate([src, np.arange(N, dtype=np.int64)])
    dst_all = np.concatenate([dst, np.arange(N, dtype=np.int64)])

    # densify adjacency into per-core wave-major A blocks:
    # row (w*NSB + sb)*128 + srel ; col (ktile%5)*128 + drel  (w = ktile//5)
    core_of = dst_all // NCN
    ktile = (dst_all % NCN) // P
    drel = (dst_all % NCN) % P
    sblk = src_all // P
    srel = src_all % P

    a_blocks = []
    lin = (((ktile // 5) * NSB + sblk) * P + srel) * (5 * P) \
        + (ktile % 5) * P + drel
    nblk_lin = 2 * NSB * P * 5 * P
    for c in range(NCORES):
        m = core_of == c
        counts = np.bincount(lin[m], minlength=nblk_lin)
        a_blocks.append(counts.astype(np.float16).reshape(2 * NSB * P, 5 * P))

    Wcomb = np.ascontiguousarray((np.asarray(W_gcn, np.float32)
                                  @ np.asarray(W_fc3, np.float32))
                                 .astype(np.float16))
    bias16 = (np.asarray(b_gcn, np.float32) @ np.asarray(W_fc3, np.float32)
              + np.asarray(b_fc3, np.float32))
    bout_t = np.ascontiguousarray(np.tile(bias16, (P, T)).astype(np.float32))
    bgv = (np.asarray(b_ih, np.float32) + np.asarray(b_hh, np.float32))
    bg_t = np.ascontiguousarray(bgv.reshape(4, P).T.astype(np.float32))
    b2_t = np.ascontiguousarray(np.asarray(b_fc2, np.float32).reshape(P, 1))
    wih_t = np.ascontiguousarray(np.asarray(W_ih, np.float32).T.astype(np.float16))
    whh_t = np.ascontiguousarray(np.asarray(W_hh, np.float32).T.astype(np.float16))
    wfc2_t = np.ascontiguousarray(np.asarray(W_fc2, np.float32))

    in_maps = []
    for c in range(NCORES):
        sl = slice(c * NCN, (c + 1) * NCN)
        dv = dinv[sl]
        dinv_t = np.zeros((P, NTILES), np.float32)
        for k in range(NTILES):
            rows = min(P, NCN - k * P)
            dinv_t[:rows, k] = dv[k * P:k * P + rows]
        in_maps.append({
            "zT": np.ascontiguousarray(z[sl].T),
            "xm": np.ascontiguousarray(
                x_mask[sl].reshape(NCN, TNF).astype(np.uint8)),
            "wfc2": wfc2_t,
            "b2": b2_t,
            "wih": wih_t,
            "whh": whh_t,
            "bg": bg_t,
            "wcomb": Wcomb,
            "bout": bout_t,
            "dinvt": dinv_t,
            "ablk": a_blocks[c],
        })
    return in_maps


def kernel(z, edge_index, x_mask, W_fc2, b_fc2, W_ih, W_hh, b_ih, b_hh,
           W_gcn, b_gcn, W_fc3, b_fc3):
    global LAST_RESULTS
    in_maps = _preprocess(z, edge_index, x_mask, W_fc2, b_fc2,
                          W_ih, W_hh, b_ih, b_hh,
                          W_gcn, b_gcn, W_fc3, b_fc3)
    if "nc" not in _BUILD_CACHE:
        _BUILD_CACHE["nc"] = _build()
    nc = _BUILD_CACHE["nc"]

    trace = bool(int(os.environ.get("KERNEL_TRACE", "0")))
    res = bass_utils.run_bass_kernel_spmd(
        nc, in_maps, core_ids=list(range(NCORES)), trace=trace)
    LAST_RESULTS = res

    out = np.empty((N, T, NF), np.float32)
    for c in range(NCORES):
        out[c * NCN:(c + 1) * NCN] = res.results[c]["xhat"].reshape(NCN, T, NF)
    return out



# revision 14
# speedup vs baseline: 1.5062x; 1.5062x over previous
# Trainium2 Bass kernel for nn_Decoder (LSTM decoder + GCN message passing).
#
# Strategy (8 NeuronCores, SPMD), v4 — dst-partitioned GCN, AG under LSTM:
#   * Data-parallel over nodes N=10000 -> 1250 nodes/core for fc2 + LSTM +
#     projection. State feature-major ([H, nodes]); all elementwise LSTM
#     math in fp16 SBUF, activations on the Scalar engine (per-step
#     bottleneck: 5 x [128,1250] activations).
#   * Algebraic rewrite: GCN aggregation and fc3 are both linear, so
#     aggregate AFTER projecting features to NF=16 with W_gcn@W_fc3.
#     The per-node mask*dinv[src] scale is folded into a pre-scaled copy
#     of h (hm = h * mdv) so the projected Y needs no post-processing.
#   * Y rows AllGather'ed in 4 column slices DURING the LSTM (t0-5 after
#     step 6, t6-8 after step 9, t9-10 after step 11, t11 at the end) so
#     only a 16-column AG remains on the critical path — the collective
#     and its first-sync skew hide under the LSTM.
#   * GCN is dst-partitioned: each core owns 1250 dst rows and sums over
#     all 79 source blocks with fp8e4 A-blocks (exact edge multiplicities)
#     against the fp16 gathered Y table: agg[k] = sum_sb A[sb,k].T@Y[sb].
#     Output = agg*dinv[dst] + bias, straight from PSUM — no collective
#     after the compute.
import os
import numpy as np

import concourse.bass as bass
import concourse.bacc as bacc
import concourse.tile as tile
from concourse import mybir
from concourse import bass_utils

P = 128
N, T, NF, H, L, E = 10000, 12, 16, 128, 64, 160000
NCORES = 8
NCN = N // NCORES            # 1250 nodes per core
NSB = NCN // P + 1           # 10 own dst tiles (last has 98 rows)
NJ = (N + P - 1) // P        # 79 global src blocks (last has 16 rows)
TNF = T * NF                 # 192
CH = [(0, 512), (512, 512), (1024, NCN - 1024)]  # matmul col chunks (<=512)
NPAD = NSB * P               # 1280 padded own-node columns

# Y AllGather slices: (first t, n timesteps), shipped after LSTM step
# (first t + n) completes; the last one-timestep slice is the only AG on
# the critical path.
SLICES = [(0, 6), (6, 3), (9, 2), (11, 1)]

F32 = mybir.dt.float32
F16 = mybir.dt.float16
F8 = mybir.dt.float8e4

GATE_FUNCS = ["Sigmoid", "Sigmoid", "Tanh", "Sigmoid"]  # i, f, g, o

_BUILD_CACHE = {}
LAST_RESULTS = None  # BassKernelResults of the most recent run (for test harness)


def _build():
    nc = bacc.Bacc("TRN2", target_bir_lowering=False, debug=False,
                   num_devices=NCORES)

    # ---------------- I/O declarations ----------------
    zT = nc.dram_tensor("zT", [L, NCN], F16, kind="ExternalInput")
    wfc2 = nc.dram_tensor("wfc2", [L, H], F16, kind="ExternalInput")
    b2 = nc.dram_tensor("b2", [P, 1], F32, kind="ExternalInput")
    wih = nc.dram_tensor("wih", [H, 4 * H], F16, kind="ExternalInput")
    whh = nc.dram_tensor("whh", [H, 4 * H], F16, kind="ExternalInput")
    bg = nc.dram_tensor("bg", [P, 4], F32, kind="ExternalInput")
    wcomb = nc.dram_tensor("wcomb", [H, NF], F16, kind="ExternalInput")
    bout = nc.dram_tensor("bout", [P, TNF], F32, kind="ExternalInput")
    mdvrow = nc.dram_tensor("mdvrow", [P, NCN], F16, kind="ExternalInput")
    dinvt = nc.dram_tensor("dinvt", [P, NSB], F32, kind="ExternalInput")
    # A-blocks: [srel, k*79*128 + sb*128 + drel], fp8 multiplicities.
    ablk = nc.dram_tensor("ablk", [P, NSB * NJ * P], F8, kind="ExternalInput")
    xhat = nc.dram_tensor("xhat", [NCN, TNF], F32, kind="ExternalOutput")

    with tile.TileContext(nc) as tc:
        with tc.tile_pool(name="cpool", bufs=1) as cp, \
             tc.tile_pool(name="spool", bufs=1) as sp, \
             tc.tile_pool(name="wpA", bufs=4) as wpA, \
             tc.tile_pool(name="dram", bufs=1, space="DRAM") as dp:

            # ---- constant loads ----
            zt_sb = cp.tile([L, NCN], F16)
            nc.sync.dma_start(zt_sb[:], zT[:])
            wfc2_sb = cp.tile([L, H], F16)
            nc.sync.dma_start(wfc2_sb[:], wfc2[:])
            b2_sb = cp.tile([P, 1], F32)
            nc.sync.dma_start(b2_sb[:], b2[:])
            wih_sb = cp.tile([H, 4 * H], F16)
            nc.sync.dma_start(wih_sb[:], wih[:])
            whh_sb = cp.tile([H, 4 * H], F16)
            nc.sync.dma_start(whh_sb[:], whh[:])
            bg_sb = cp.tile([P, 4], F32)
            nc.sync.dma_start(bg_sb[:], bg[:])
            wcomb_sb = cp.tile([H, NF], F16)
            nc.sync.dma_start(wcomb_sb[:], wcomb[:])
            bout_sb = cp.tile([P, TNF], F32)
            nc.sync.dma_start(bout_sb[:], bout[:])
            mdv_sb = cp.tile([P, NCN], F16)
            nc.sync.dma_start(mdv_sb[:], mdvrow[:])
            dinv_sb = cp.tile([P, NSB], F32)
            nc.sync.dma_start(dinv_sb[:], dinvt[:])

            # AllGather staging (per column slice)
            yshard_s = [dp.tile([NCN, n * NF], F16, name=f"yshard{i}")
                        for i, (t0, n) in enumerate(SLICES)]
            yfull_s = [dp.tile([N, n * NF], F16, addr_space="Shared",
                               name=f"yfull{i}")
                       for i, (t0, n) in enumerate(SLICES)]

            # A-block chunks (one own dst tile each): prefetch 4 during LSTM
            PREF = 4
            abc_tiles = []
            for k in range(NSB):
                abc = wpA.tile([P, NJ * P], F8, name=f"abc{k}",
                               tag="abc", bufs=PREF)
                if k < PREF:
                    nc.sync.dma_start(abc[:],
                                      ablk[:, k * NJ * P:(k + 1) * NJ * P])
                abc_tiles.append(abc)

            # Own Y rows (pre-scaled): [srel, k*192 + t*16 + f]
            ysb = sp.tile([P, NSB * TNF], F16, name="ysb")
            # Gathered Y table: [srel, sb*192 + t*16 + f]
            ytab = sp.tile([P, NJ * TNF], F16, name="ytab")
            nc.vector.memset(ytab[:, (NJ - 1) * TNF:], 0.0)

            def ship_slice(i):
                t0, ns = SLICES[i]
                c0, w = t0 * NF, ns * NF
                # own shard -> DRAM (two DMAs: 9 full tiles + 98-row tail)
                nc.sync.dma_start(
                    yshard_s[i][:9 * P, :].rearrange("(k p) f -> p k f", p=P),
                    ysb[:].rearrange("p (k f) -> p k f", f=TNF)
                    [:, :9, c0:c0 + w])
                nc.sync.dma_start(
                    yshard_s[i][9 * P:, :],
                    ysb[:NCN - 9 * P, 9 * TNF + c0:9 * TNF + c0 + w])
                nc.gpsimd.collective_compute(
                    "AllGather", mybir.AluOpType.bypass,
                    replica_groups=[list(range(NCORES))],
                    ins=[yshard_s[i].opt()], outs=[yfull_s[i].opt()])
                # assemble into ytab (gpsimd queue: only AGs live there)
                nc.gpsimd.dma_start(
                    ytab[:].rearrange("p (sb f) -> p sb f", f=TNF)
                    [:, :NJ - 1, c0:c0 + w],
                    yfull_s[i][:(NJ - 1) * P, :].rearrange(
                        "(sb p) f -> p sb f", p=P))
                nc.gpsimd.dma_start(
                    ytab[:N - (NJ - 1) * P,
                         (NJ - 1) * TNF + c0:(NJ - 1) * TNF + c0 + w],
                    yfull_s[i][(NJ - 1) * P:, :])

            # ---- hd = z @ W_fc2 + b_fc2 (feature-major: hdT [H, nodes]) ----
            hdT = sp.tile([H, NCN], F16)
            with tc.tile_pool(name="psI", bufs=1, space="PSUM") as psI:
                pf = psI.tile([P, NCN], F32)
                for off, sz in CH:
                    nc.tensor.matmul(out=pf[:, off:off + sz], lhsT=wfc2_sb[:],
                                     rhs=zt_sb[:, off:off + sz],
                                     start=True, stop=True)
                nc.scalar.activation(
                    out=hdT[:], in_=pf[:],
                    func=mybir.ActivationFunctionType.Identity,
                    bias=b2_sb[:, :1])

            # ---- LSTM (T steps, feature-major fp16 state) ----
            cstate = sp.tile([P, NCN], F16)
            nc.vector.memset(cstate[:], 0.0)
            h_pp = [sp.tile([P, NCN], F16, name=f"hbuf{i}") for i in range(2)]
            hm_pp = [sp.tile([P, NPAD], F16, name=f"hmbuf{i}")
                     for i in range(2)]
            nc.vector.memset(hm_pp[0][:, NCN:], 0.0)
            nc.vector.memset(hm_pp[1][:, NCN:], 0.0)

            with tc.tile_pool(name="psG", bufs=1, space="PSUM") as psG, \
                 tc.tile_pool(name="psY", bufs=1, space="PSUM") as psY, \
                 tc.tile_pool(name="wpL", bufs=1) as wpL:

                def emit_proj(t):
                    # Y[:, :, t] = (hm_t.T @ wcomb) for all 10 own tiles
                    hm = hm_pp[t % 2]
                    py = psY.tile([P, NSB * NF], F32, name=f"py{t}",
                                  tag="py", bufs=2)
                    for k in range(NSB):
                        nc.tensor.matmul(out=py[:, k * NF:(k + 1) * NF],
                                         lhsT=hm[:, k * P:(k + 1) * P],
                                         rhs=wcomb_sb[:],
                                         start=True, stop=True)
                    return py

                def emit_proj_copy(t, py):
                    nc.vector.tensor_copy(
                        out=ysb[:].rearrange("p (k f) -> p k f", f=TNF)
                        [:, :, t * NF:(t + 1) * NF],
                        in_=py[:].rearrange("p (k f) -> p k f", f=NF))

                ship_after = {t0 + ns: i
                              for i, (t0, ns) in enumerate(SLICES[:-1])}
                py_prev = None
                for t in range(T):
                    prev = hdT if t == 0 else h_pp[(t - 1) % 2]
                    sg = {}
                    first = True
                    for q in (1, 0, 2, 3):   # f, i, g, o
                        wsl = slice(q * H, (q + 1) * H)
                        pq = psG.tile([P, NCN], F32, name=f"pq{t}_{q}",
                                      tag="pq", bufs=2)
                        for off, sz in CH:
                            nc.tensor.matmul(out=pq[:, off:off + sz],
                                             lhsT=wih_sb[:, wsl],
                                             rhs=hdT[:, off:off + sz],
                                             start=True, stop=False)
                        for off, sz in CH:
                            nc.tensor.matmul(out=pq[:, off:off + sz],
                                             lhsT=whh_sb[:, wsl],
                                             rhs=prev[:, off:off + sz],
                                             start=False, stop=True)
                        if first and t > 0:
                            py_prev = emit_proj(t - 1)  # fills PE idle slot
                        first = False
                        sg[q] = wpL.tile([P, NCN], F16, name=f"sg{t}_{q}",
                                         tag=f"sg{q}", bufs=2)
                        nc.scalar.activation(
                            out=sg[q][:], in_=pq[:],
                            func=getattr(mybir.ActivationFunctionType,
                                         GATE_FUNCS[q]),
                            bias=bg_sb[:, q:q + 1])
                        if q == 1:
                            nc.vector.tensor_mul(out=cstate[:], in0=cstate[:],
                                                 in1=sg[1][:])
                        elif q == 2:
                            tmp = wpL.tile([P, NCN], F16, name=f"tmp{t}",
                                           tag="tmp", bufs=2)
                            nc.vector.tensor_mul(out=tmp[:], in0=sg[0][:],
                                                 in1=sg[2][:])
                            nc.vector.tensor_add(out=cstate[:], in0=cstate[:],
                                                 in1=tmp[:])
                            if t > 0:
                                emit_proj_copy(t - 1, py_prev)
                    thc = wpL.tile([P, NCN], F16, name=f"thc{t}",
                                   tag="thc", bufs=2)
                    nc.scalar.activation(
                        out=thc[:], in_=cstate[:],
                        func=mybir.ActivationFunctionType.Tanh)
                    nc.vector.tensor_mul(out=h_pp[t % 2][:],
                                         in0=sg[3][:], in1=thc[:])
                    nc.vector.tensor_mul(out=hm_pp[t % 2][:, :NCN],
                                         in0=h_pp[t % 2][:], in1=mdv_sb[:])
                    # ship a finished column slice (copies for t-1 are in)
                    if t in ship_after:
                        ship_slice(ship_after[t])
                py_prev = emit_proj(T - 1)
                emit_proj_copy(T - 1, py_prev)
                ship_slice(len(SLICES) - 1)

            # ---- GCN: own dst tiles from gathered Y ----
            with tc.tile_pool(name="psC", bufs=1, space="PSUM") as psC, \
                 tc.tile_pool(name="wpF", bufs=1) as wpF:
                for k in range(NSB):
                    abc = abc_tiles[k]
                    if k >= PREF:
                        nc.sync.dma_start(
                            abc[:], ablk[:, k * NJ * P:(k + 1) * NJ * P])
                    pa = psC.tile([P, TNF], F32, name=f"pa{k}",
                                  tag="pa", bufs=2)
                    for sb in range(NJ):
                        nc.tensor.matmul(
                            out=pa[:],
                            lhsT=abc[:, sb * P:(sb + 1) * P],
                            rhs=ytab[:, sb * TNF:(sb + 1) * TNF],
                            start=(sb == 0), stop=(sb == NJ - 1))
                    rows = min(P, NCN - k * P)
                    xo = wpF.tile([P, TNF], F32, name=f"xo{k}",
                                  tag="xo", bufs=2)
                    nc.vector.tensor_scalar(
                        out=xo[:], in0=pa[:],
                        scalar1=dinv_sb[:, k:k + 1], scalar2=None,
                        op0=mybir.AluOpType.mult)
                    nc.vector.tensor_add(out=xo[:], in0=xo[:], in1=bout_sb[:])
                    nc.scalar.dma_start(xhat[k * P:k * P + rows, :],
                                        xo[:rows])

    nc.compile()
    return nc


def _preprocess(z, edge_index, x_mask, W_fc2, b_fc2, W_ih, W_hh, b_ih, b_hh,
                W_gcn, b_gcn, W_fc3, b_fc3):
    f8np = mybir.dt.np(F8)
    z = np.asarray(z, np.float32)
    edge_index = np.asarray(edge_index).astype(np.int64)
    x_mask = np.asarray(x_mask)
    src = edge_index[0]
    dst = edge_index[1]
    deg = np.bincount(dst, minlength=N) + 1.0
    dinv = (1.0 / np.sqrt(deg)).astype(np.float32)
    node_mask = x_mask.reshape(N, T * NF).any(axis=1)
    mdv = dinv * node_mask.astype(np.float32)

    src_all = np.concatenate([src, np.arange(N, dtype=np.int64)])
    dst_all = np.concatenate([dst, np.arange(N, dtype=np.int64)])

    # fp8 A-blocks, dst-partitioned: core = dst // 1250;
    # [srel, k*79*128 + sb*128 + drel], k = own dst tile, sb = global
    # source block.
    core_of = dst_all // NCN
    k = (dst_all % NCN) // P
    drel = (dst_all % NCN) % P
    sb = src_all // P
    srel = src_all % P
    lin = srel * (NSB * NJ * P) + k * (NJ * P) + sb * P + drel
    a_blocks = []
    for c in range(NCORES):
        m = core_of == c
        counts = np.bincount(lin[m], minlength=P * NSB * NJ * P)
        a_blocks.append(counts.astype(f8np).reshape(P, NSB * NJ * P))

    Wcomb = np.ascontiguousarray((np.asarray(W_gcn, np.float32)
                                  @ np.asarray(W_fc3, np.float32))
                                 .astype(np.float16))
    bias16 = (np.asarray(b_gcn, np.float32) @ np.asarray(W_fc3, np.float32)
              + np.asarray(b_fc3, np.float32))
    bout_t = np.ascontiguousarray(np.tile(bias16, (P, T)).astype(np.float32))
    bgv = (np.asarray(b_ih, np.float32) + np.asarray(b_hh, np.float32))
    bg_t = np.ascontiguousarray(bgv.reshape(4, P).T.astype(np.float32))
    b2_t = np.ascontiguousarray(np.asarray(b_fc2, np.float32).reshape(P, 1))
    wih_t = np.ascontiguousarray(
        np.asarray(W_ih, np.float32).T.astype(np.float16))
    whh_t = np.ascontiguousarray(
        np.asarray(W_hh, np.float32).T.astype(np.float16))
    wfc2_t = np.ascontiguousarray(
        np.asarray(W_fc2, np.float32).astype(np.float16))

    def per_tile(vec):
        out = np.zeros((P, NSB), np.float32)
        for kk in range(NSB):
            rows = min(P, NCN - kk * P)
            out[:rows, kk] = vec[kk * P:kk * P + rows]
        return out

    in_maps = []
    for c in range(NCORES):
        sl = slice(c * NCN, (c + 1) * NCN)
        in_maps.append({
            "zT": np.ascontiguousarray(z[sl].T.astype(np.float16)),
            "wfc2": wfc2_t,
            "b2": b2_t,
            "wih": wih_t,
            "whh": whh_t,
            "bg": bg_t,
            "wcomb": Wcomb,
            "bout": bout_t,
            "mdvrow": np.ascontiguousarray(
                np.broadcast_to(mdv[sl].astype(np.float16), (P, NCN))),
            "dinvt": per_tile(dinv[sl]),
            "ablk": a_blocks[c],
        })
    return in_maps


def kernel(z, edge_index, x_mask, W_fc2, b_fc2, W_ih, W_hh, b_ih, b_hh,
           W_gcn, b_gcn, W_fc3, b_fc3):
    global LAST_RESULTS
    in_maps = _preprocess(z, edge_index, x_mask, W_fc2, b_fc2,
                          W_ih, W_hh, b_ih, b_hh,
                          W_gcn, b_gcn, W_fc3, b_fc3)
    if "nc" not in _BUILD_CACHE:
        _BUILD_CACHE["nc"] = _build()
    nc = _BUILD_CACHE["nc"]

    trace = bool(int(os.environ.get("KERNEL_TRACE", "0")))
    res = bass_utils.run_bass_kernel_spmd(
        nc, in_maps, core_ids=list(range(NCORES)), trace=trace)
    LAST_RESULTS = res

    out = np.empty((N, T, NF), np.float32)
    for c in range(NCORES):
        out[c * NCN:(c + 1) * NCN] = res.results[c]["xhat"].reshape(NCN, T, NF)
    return out
